# revision 83
# baseline (speedup 1.0000x reference)
"""Trainium2 Bass kernel for nn_ComplexMultiheadAttention.

Model (B=2, L=4096, E=512, H=8, D=64, W=128):
  qr,qi = query @ qWr.T + qbr, query @ qWi.T + qbi   (same k; v real part only)
  scores = (qr@kr^T + qi@ki^T) / sqrt(D)             per (b, h)
  mask: position i may attend j iff j >= i - W  (no causal mask)
  probs = softmax(scores);  o = probs @ vr
  out_r = o @ oWr.T + obr;  out_i = o @ oWi.T + obi   -> returns (out_r, out_i)

Sharding: 16 (b, h) units over 8 cores -> each core gets one b and two
adjacent heads (hA, hB). Host pre-transposes q/k/v to [E, L] per b, casts
to bf16, and slices per-head weight blocks. Each core computes a partial
[L, 2E] output in bf16 (its heads' contribution); host sums the 4 partials
per b in f32 and adds the output biases (including the folded V-bias term
vb @ oW.T, so the device kernel never applies the V projection bias).

On-chip per core, a SLAB PIPELINE over 512-col slabs processed from the
sequence tail (attention quarter q needs Kc/vr blocks >= 4q-1, i.e.
slabs q-1..7):

  L7 P7 L6 P6 [A7 P5] [A6 P4] ... [A2 P0] A1 A0 flush

  load L(s):  one strided DMA per tensor brings the whole slab (all four
              e-tile planes) -- few, large DMAs keep the in-order SP queue
              free for the later onT transposes and output stores.
  proj P(s):  Qc_h/Kc_h = [re; im]^T [128, L] projections (PE, contraction
              over 4 e-tiles; both heads share a PSUM tile; evacuation
              + bias on whichever of ScalarE/DVE is less loaded).  V is
              projected TRANSPOSED (out [l, vd]) so it lands directly in
              the [j, d] layout PV needs (ones column at d=64 for softmax
              row-sums); two l-tiles share a tile and evacuate together.
  attn A(q):  per head, j-blocks processed as UNITS: boundary blocks
              (banded-mask tril) stay single at exact width; interior
              blocks pair up so one exp op covers 1024 cols; narrow
              boundary units are interleaved between wide pairs so the
              3-deep score-PSUM ring always holds enough queued PE work
              to cover the exp latency.  exp runs on ScalarE (native Exp)
              or DVE (Schraudolph bf16 bit-trick: round(s*A+B) as int16
              bitcast to bf16), chosen by a running load balancer with
              the DVE share capped (SCH_CAP) for accuracy.  DVE boundary
              exps fuse the mask via scalar_tensor_tensor with an
              additive right-aligned [ones|tril] window whose masked
              entries saturate the int16 round to -32768 -> -0.0 bf16;
              ScalarE boundary exps get a multiplicative tril mask on the
              otherwise-idle Pool engine (GPSIMD cannot touch PSUM, so
              all PSUM-reading elementwise work is ScalarE/DVE only).
              PV accumulates TRANSPOSED: oT[i-tile, 65] += pt[j, i]^T @
              vr[j, 65], deferred PV_DEFER blocks behind the scores; the
              first PV matmul per (q, head) carries start=True (PSUM bank
              zero fill), the last carries stop.  Normalization: one
              batched DVE reciprocal of the 4 row-sum columns, then
              per-i-tile multiplies (balanced S/D).
  out:        per quarter, onT [i, d] -> onorm [d, i] via DMA xbar
              transposes issued per-i-tile as head B's norms complete
              (PE-transpose via identity permutation for the last
              quarter, where no later work hides the DMA latency), then
              out = onorm^T @ [oWr|oWi] in [128, 512] chunks dripped one
              per unit into the next quarter's PE stream, evacuated to
              bf16 (balanced S/D) and DMA'd out.
"""

import numpy as np
import ml_dtypes
import orjson

import concourse.bass as bass
import concourse.mybir as mybir
import concourse.tile as tile
from concourse.bass_utils import run_bass_kernel_spmd
from concourse.vector_clock import ScopedClock

F32 = mybir.dt.float32
BF16 = mybir.dt.bfloat16
I16 = mybir.dt.int16
BF = ml_dtypes.bfloat16

B, L, E, H, D, W = 2, 4096, 512, 8, 64, 128
NBLK = L // 128          # 32 j-blocks
NQ = 8                   # query quarters (512 cols each)
QCOLS = L // NQ          # 512
ET = E // 128            # 4 contraction e-tiles
LC = L // 512            # 8 projection l-chunks

# Schraudolph exp for bf16 out: bits16 = round(s * A + B), bitcast to bf16
# approximates exp(s * 0.125). A folds the 1/sqrt(D) score scale.
A_SCH = 0.125 * 1.4426950408889634 * 128.0
B_SCH = 127.0 * 128.0 - 5.5
NEGV = -1.0e6            # B+NEGV saturates the int16 round -> -0.0 bf16

# engine-balance cost model (ns), from the TRN2 instruction cost model
SCH_CAP = 0.48           # max fraction of exp free-size via DVE Schraudolph
PV_DEFER = 10           # j-blocks of score lookahead before each PV
SPS_BUFS = 3             # score PSUM tiles ([128,1024] f32 = 2 banks each)
FPS_BUFS = 1             # phase-3 chunk PSUM tiles (1 bank each)
OTP_BUFS = 1             # oT accumulator PSUM tiles (1 bank each)
EXP_SPLIT = False        # split pair exps into two 512-wide half ops
S_RESERVE = 0.0          # extra cost bias pushing non-exp ops off ScalarE
EXP_OVERRIDE = {}        # exp index -> engine, from sim-guided tuning
DRIP_FIRST = False       # p3 chunk before (True) or after (False) PV pops
USE_FPS_BND = False      # boundary-single score tiles borrow the fps bank


def _cS(free):
    return free * 0.8333 + 185.0


def _cD(free, init=125.0):
    return free * 1.0417 + init


def _cP(free, eff=0.6):
    return free * 0.8333 / eff + 95.0


class _Bal:
    def __init__(self):
        self.t = {"S": 0.0, "D": 0.0, "P": 0.0}

    def pick(self, costs):
        e = min(costs, key=lambda k: self.t[k] + costs[k])
        self.t[e] += costs[e]
        return e


# ---------------------------------------------------------------------------
# Workaround: this walrus build rejects instructions carrying >1 sem wait on
# the TileContext tail drain. Spill extra waits onto standalone wait_ge ops.
def _patched_drain_and_barrier(self, tick_clock, wait_clock):
    nc = self.nc
    drain_inst = nc.sync.drain()
    wait_clock.add_sem_waits(
        drain_inst.ins, ScopedClock({None: tick_clock.global_clock})
    )
    si = drain_inst.ins.sync_info
    if si is not None and len(si.on_wait) > 1:
        waits = list(si.on_wait)
        si.on_wait = waits[:1]
        drain_inst.ins.sync_info = si
        id_to_handle = {h.num: h for h in self.sems.allocated().values()}
        for w in waits[1:]:
            nc.sync.wait_ge(id_to_handle[w.id], w.wait_value)
    nc.all_engine_barrier()
    popped = nc._tile_sem_poison_stack.pop()
    assert popped is self._sem_poison
    nc.clear_and_free_semaphores(list(self.sems.allocated().values()))
    nc.all_engine_barrier()


tile.TileContext._drain_and_barrier = _patched_drain_and_barrier


def _split_bir_waits(data, cap=1):
    """This walrus build rejects >cap sem waits on one instruction; hoist
    extras onto wait-only EventSemaphore instructions inserted just before
    (same engine, same stream position -> identical semantics)."""
    n = 0
    for fn in data["functions"]:
        for bb in fn["blocks"]:
            out = []
            for inst in bb["instructions"]:
                si = inst.get("sync_info")
                if si:
                    ws = si.get("on_wait") or []
                    if len(ws) > cap:
                        for w in ws[:-cap]:
                            n += 1
                            out.append({
                                "debug": inst.get("debug", 0),
                                "engine": inst["engine"],
                                "ins": [], "outs": [],
                                "name": f"sw-{n}-{inst['name']}",
                                "opcode": "EventSemaphore",
                                "sync_info": {"on_update": [],
                                              "on_wait": [w]},
                            })
                        si["on_wait"] = ws[-cap:]
                out.append(inst)
            bb["instructions"] = out
    return data
# ---------------------------------------------------------------------------


def _ic(qs, jb):
    """Highest i (quarter-local, exclusive) allowed to attend j-block jb."""
    return min(QCOLS, (jb + 2) * 128 - qs)


def build_program():
    nc = bass.Bass("TRN2", target_bir_lowering=False, debug=False)

    qT = nc.dram_tensor("qT", [E, L], BF16, kind="ExternalInput")
    kT = nc.dram_tensor("kT", [E, L], BF16, kind="ExternalInput")
    vT = nc.dram_tensor("vT", [E, L], BF16, kind="ExternalInput")
    # weight inputs arrive pre-transposed to the SBUF layout [128, E]:
    # row p, col et*128+m  ==  W[et*128+p, m]
    WqA = nc.dram_tensor("WqA", [128, E], BF16, kind="ExternalInput")
    WqB = nc.dram_tensor("WqB", [128, E], BF16, kind="ExternalInput")
    WkA = nc.dram_tensor("WkA", [128, E], BF16, kind="ExternalInput")
    WkB = nc.dram_tensor("WkB", [128, E], BF16, kind="ExternalInput")
    Wv = nc.dram_tensor("Wv", [128, E], BF16, kind="ExternalInput")
    Wo = nc.dram_tensor("Wo", [128, 2 * E], BF16, kind="ExternalInput")
    bqA = nc.dram_tensor("bqA", [128, 1], F32, kind="ExternalInput")
    bqB = nc.dram_tensor("bqB", [128, 1], F32, kind="ExternalInput")
    bkA = nc.dram_tensor("bkA", [128, 1], F32, kind="ExternalInput")
    bkB = nc.dram_tensor("bkB", [128, 1], F32, kind="ExternalInput")
    maskbuf = nc.dram_tensor("maskbuf", [128, 768], BF16, kind="ExternalInput")
    mask3 = nc.dram_tensor("mask3", [128, 512], F32, kind="ExternalInput")
    out = nc.dram_tensor("out", [L, 2 * E], BF16, kind="ExternalOutput")

    bal = _Bal()
    exp_idx = [0]
    exp_log = build_program.exp_log = []
    exp_sch_free = [0]
    exp_tot_free = [0]

    with tile.TileContext(nc) as tc:
        with tc.tile_pool(name="persist", bufs=1) as persist:
            # ---- persistent weights / constants -----------------------------
            w_sb = {}
            ws = persist.tile([128, E], BF16, tag="WqA", name="WqA")
            nc.sync.dma_start(ws[:, :], WqA.ap())
            w_sb["WqA"] = ws
            b_sb = {}
            for name, t in [("bqA", bqA), ("bqB", bqB), ("bkA", bkA),
                            ("bkB", bkB)]:
                bs = persist.tile([128, 1], F32, tag=name, name=name)
                nc.sync.dma_start(bs[:, :], t.ap())
                b_sb[name] = bs

            # ---- persistent activations -------------------------------------
            Qc = {h: persist.tile([128, L], BF16, tag=f"Qc{h}", name=f"Qc{h}")
                  for h in "AB"}
            Kc = {h: persist.tile([128, L], BF16, tag=f"Kc{h}", name=f"Kc{h}")
                  for h in "AB"}
            # vr: head A blocks at cols [0, 2080), head B at [2080, 4160);
            # block jb occupies 65 cols (64 v-dims + ones col for row-sums)
            vr_sb = persist.tile([128, 2 * NBLK * 65], BF16, tag="vr")
            ones_ap = vr_sb[:, :].rearrange(
                "p (h b c) -> p h b c", h=2, c=65)[:, :, :, 64:65]
            nc.gpsimd.memset(ones_ap, 1.0)

            # ---- interleaved projections + attention ------------------------
            # The second L-half (cols 2048:4096) of q/k/v is projected
            # first; attention quarters 7,6,5 touch only Kc/vr blocks >= 19
            # and Qc cols >= 2560, so they run while the first half's DMA
            # stream lands; then the first half projects and quarters 4..0
            # follow.  Projections share the score PSUM pool (8 banks total).
            with (
                tc.tile_pool(name="xt", bufs=1) as xt,
                tc.tile_pool(name="sps", bufs=SPS_BUFS, space="PSUM") as sps,
                tc.tile_pool(name="oTp", bufs=OTP_BUFS, space="PSUM") as oTp,
                tc.tile_pool(name="fps", bufs=FPS_BUFS, space="PSUM") as fps,
                tc.tile_pool(name="ptp", bufs=PV_DEFER + 4) as ptp,
                tc.tile_pool(name="onTp", bufs=2) as onTp,
                tc.tile_pool(name="onqTp", bufs=2) as onqTp,
                tc.tile_pool(name="osp", bufs=4) as osp,
                tc.tile_pool(name="recp", bufs=8) as recp,
            ):
                pending_p3 = []
                flush_alt = [0]
                # one [128, 4*L] tile per tensor: e-tile-major planes so a
                # whole 512-col slab of all four e-tiles moves in ONE DMA
                # (the SP queue issues DMAs serially at ~565ns each; per-et
                # DMAs would flood it and delay the later transposes)
                xts = {}
                xv3 = {}
                for s, src in (("q", qT), ("k", kT), ("v", vT)):
                    t = xt.tile([128, ET * L], BF16, tag=f"x{s}",
                                name=f"x{s}")
                    xts[s] = t
                    xv3[s] = t[:, :].rearrange("p (e l) -> p e l", e=ET)

                def load_slab(sq, tensors=("q", "k", "v"), et_split=1):
                    """DMA the 512-col slab sq of the given tensors (all
                    e-tiles in one strided DMA each; et_split>1 splits the
                    e-tile planes across that many DMAs so the first
                    projection matmuls can start before the whole slab
                    lands)."""
                    lo = sq * QCOLS
                    srcs = {"q": qT, "k": kT, "v": vT}
                    for s in tensors:
                        src3 = srcs[s].ap().rearrange(
                            "(e p) l -> p e l", e=ET)
                        step = ET // et_split
                        for e0 in range(0, ET, step):
                            nc.sync.dma_start(
                                xv3[s][:, e0:e0 + step, lo:lo + QCOLS],
                                src3[:, e0:e0 + step, lo:lo + QCOLS])

                # NOTE: GPSIMD/Pool cannot access PSUM, so every PSUM-
                # touching elementwise op balances across ScalarE/DVE only;
                # Pool gets the SBUF-side multiplicative masks.
                def emit_evac_bias(dst_ap, src_ap, bias_ap, free):
                    e = bal.pick({"S": _cS(free) + S_RESERVE,
                                  "D": _cD(free)})
                    if e == "S":
                        nc.scalar.activation(
                            dst_ap, src_ap,
                            mybir.ActivationFunctionType.Identity,
                            bias=bias_ap)
                    else:
                        nc.vector.tensor_scalar(
                            dst_ap, src_ap, bias_ap, None,
                            mybir.AluOpType.add)

                def emit_copy(dst_ap, src_ap, free):
                    e = bal.pick({"S": _cS(free) + S_RESERVE,
                                  "D": _cD(free)})
                    if e == "S":
                        nc.scalar.copy(dst_ap, src_ap)
                    else:
                        nc.vector.tensor_copy(dst_ap, src_ap)

                def drip():
                    if pending_p3:
                        pending_p3.pop(0)()

                vr_4d = vr_sb[:, :].rearrange(
                    "p (h b c) -> p h b c", h=2, c=65)

                def proj_slab(sq):
                    """Project q, k (both heads) and v for slab sq."""
                    lo = sq * QCOLS
                    for s, targets in [
                        ("q", [("WqA", Qc["A"], "bqA"),
                               ("WqB", Qc["B"], "bqB")]),
                        ("k", [("WkA", Kc["A"], "bkA"),
                               ("WkB", Kc["B"], "bkB")]),
                    ]:
                        # both heads share one PSUM tile
                        ps = sps.tile([128, 1024], F32, tag="st", name="ps")
                        for sub, (wname, dst, bname) in enumerate(targets):
                            ws = w_sb[wname]
                            for et in range(ET):
                                nc.tensor.matmul(
                                    ps[:, sub * 512:sub * 512 + 512],
                                    ws[:, et * 128:(et + 1) * 128],
                                    xv3[s][:, et, lo:lo + 512],
                                    start=(et == 0), stop=(et == ET - 1),
                                )
                            drip()
                            emit_evac_bias(
                                dst[:, lo:lo + 512],
                                ps[:, sub * 512:sub * 512 + 512],
                                b_sb[bname][:, :], 512)
                    # V transposed projection: out [l-tile, vd] == vr
                    # layout; two l-tiles share a PSUM tile and evacuate
                    # in one op
                    wv = w_sb["Wv"]
                    for lt in range(sq * 4, sq * 4 + 4, 2):
                        vp = sps.tile([128, 1024], F32, tag="st", name="vp")
                        for sub in range(2):
                            for et in range(ET):
                                nc.tensor.matmul(
                                    vp[:, sub * 128:sub * 128 + 128],
                                    xv3["v"][:, et,
                                             (lt + sub) * 128:
                                             (lt + sub + 1) * 128],
                                    wv[:, et * 128:(et + 1) * 128],
                                    start=(et == 0), stop=(et == ET - 1),
                                )
                        emit_copy(
                            vr_4d[:, :, lt:lt + 2, 0:64],
                            vp[:, 0:256].rearrange(
                                "p (l h c) -> p h l c", l=2, h=2),
                            256)
                        drip()

                def emit_exp(st, pt, w, bnd):
                    """Exp of st[:, 0:w] -> pt[:, 0:w] bf16.  w is 512 for
                    a single interior block, 1024 for an interior pair;
                    bnd: boundary block (tril mask over [w-128, w))."""
                    in_ap = st[:, 0:w]
                    out_ap = pt[:, 0:w]
                    exp_tot_free[0] += w
                    sch_ok = exp_sch_free[0] <= SCH_CAP * 146944
                    idx = exp_idx[0]
                    exp_idx[0] += 1
                    if idx in EXP_OVERRIDE and (sch_ok
                                                or EXP_OVERRIDE[idx] == "S"):
                        # simulation-guided reassignment (see tuning loop)
                        e = EXP_OVERRIDE[idx]
                        bal.t[e] += _cS(w) if e == "S" else _cD(w)
                    else:
                        costs = {"S": _cS(w)}
                        if sch_ok:
                            costs["D"] = _cD(w)
                        e = bal.pick(costs)
                    exp_log.append((idx, e, w))
                    if e == "S":
                        nc.scalar.activation(
                            out_ap, in_ap,
                            mybir.ActivationFunctionType.Exp,
                            scale=0.125)
                        if bnd:
                            # multiplicative tril mask on Pool (SBUF-only
                            # engine; otherwise idle)
                            dst = pt[:, w - 128:w]
                            nc.gpsimd.tensor_mul(dst, dst, mask_sb[:, 0:128])
                        return
                    exp_sch_free[0] += w
                    if not bnd:
                        nc.vector.tensor_scalar(
                            out_ap.bitcast(I16), in_ap, A_SCH, B_SCH,
                            mybir.AluOpType.mult, mybir.AluOpType.add)
                        return
                    # fused additive mask: right-aligned [ones|tril] window
                    m_ap = mask3_sb[:, 512 - w:512]
                    nc.vector.scalar_tensor_tensor(
                        out_ap.bitcast(I16), in_ap, A_SCH, m_ap,
                        mybir.AluOpType.mult, mybir.AluOpType.add)

                def queue_p3(q, onqT):
                    """Queue the projection chunks to be dripped one-per-
                    group into the next quarter's PE stream (the transposes
                    were already issued per-i-tile as head B's norms
                    completed)."""
                    state = {}

                    def chunk(it, n0):
                        def emit(flush=False):
                            if n0 == 0:
                                state[it] = osp.tile(
                                    [128, 2 * E], BF16, tag="os", name="os")
                            os = state[it]
                            if flush:
                                # phase 2 is over: alternate the sps slots
                                # and the fps bank so the tail pipeline is
                                # four deep
                                slot = flush_alt[0] % 5
                                if slot == 3:
                                    fp = fps.tile([128, 512], F32,
                                                  tag="fp")
                                elif slot == 4:
                                    # the oT accumulator bank is free
                                    # once the last norms have read it
                                    fp = oTp.tile([128, 512], F32,
                                                  tag="oT", name="fpo")
                                else:
                                    fp = sps.tile([128, 512], F32,
                                                  tag="st", name="fpf")
                                flush_alt[0] += 1
                            else:
                                fp = fps.tile([128, 512], F32, tag="fp")
                            nc.tensor.matmul(
                                fp[:, :], onqT[:, it * 128:(it + 1) * 128],
                                wo_sb[:, n0:n0 + 512],
                                start=True, stop=True, skip_group_check=True)
                            emit_copy(os[:, n0:n0 + 512], fp[:, :], 512)
                            if n0 == 512:
                                lt = q * 4 + it
                                nc.sync.dma_start(
                                    out.ap()[lt * 128:(lt + 1) * 128, :],
                                    os[:, :])
                        return emit

                    for it in range(4):
                        for n0 in (0, 512):
                            pending_p3.append(chunk(it, n0))

                def attn(q):
                    qs = q * QCOLS
                    qn = qs // 128
                    jb_min = max(0, qn - 1)
                    onTq = onTp.tile([128, 512], BF16, tag="onT")
                    onqT = onqTp.tile([128, 512], BF16, tag="onqT")
                    # the PV queue is carried ACROSS the head boundary so
                    # head A's deferred tail drains under head B's score
                    # stream; each head's normalization fires as its last
                    # PV pops (so it precedes head B's first PV write to
                    # the shared oT PSUM bank)
                    pvq = []
                    norm_fns = {}

                    def pop_one():
                        fn, ph = pvq.pop(0)
                        fn()
                        if ph in norm_fns and all(x[1] != ph for x in pvq):
                            norm_fns.pop(ph)()

                    for hi, h in enumerate("AB"):
                        qch, kch = Qc[h], Kc[h]
                        jbs = list(range(jb_min, NBLK))
                        # boundary blocks stay single (exact widths, fused
                        # masks); interior blocks pair up so one exp op
                        # covers 1024 columns.  Interleave the narrow
                        # boundary units between wide pairs so the PE has
                        # enough queued work to cover the exp latency even
                        # at the head start.
                        bnd_units = []
                        i = 0
                        while (i < len(jbs)
                               and (jbs[i] + 2) * 128 - qs <= QCOLS):
                            bnd_units.append((jbs[i],))
                            i += 1
                        int_units = []
                        while i + 1 < len(jbs):
                            int_units.append((jbs[i], jbs[i + 1]))
                            i += 2
                        if i < len(jbs):
                            int_units.append((jbs[i],))
                        units = []
                        ii = 0
                        for b in bnd_units:
                            if ii < len(int_units):
                                units.append(int_units[ii])
                                ii += 1
                            units.append(b)
                        units.extend(int_units[ii:])
                        oTt = oTp.tile([128, 260], F32, tag="oT")
                        pv_first = [True]
                        pv_left = [sum(_ic(qs, jb) // 128 for jb in jbs)]

                        def emit_pv(jb, pt, k, oTt=oTt, pv_first=pv_first,
                                    pv_left=pv_left, hi=hi):
                            nit = _ic(qs, jb) // 128
                            for it in range(nit):
                                pv_left[0] -= 1
                                nc.tensor.matmul(
                                    oTt[:, it * 65:(it + 1) * 65],
                                    pt[:, k * 512 + it * 128:
                                       k * 512 + (it + 1) * 128],
                                    vr_sb[:, hi * 2080 + jb * 65:
                                          hi * 2080 + (jb + 1) * 65],
                                    start=pv_first[0],
                                    stop=(pv_left[0] == 0),
                                    skip_group_check=True,
                                )
                                pv_first[0] = False

                        def make_norm(oTt=oTt, hi=hi):
                            def norm():
                                # oT[i, 0:64] * (1/oT[i, 64]) -> onT bf16
                                rec = recp.tile([128, 4], F32, tag="rec")
                                nc.vector.reciprocal(
                                    rec[:, :].rearrange(
                                        "p (t c) -> p t c", c=1),
                                    oTt[:, :].rearrange(
                                        "p (t c) -> p t c", t=4)[:, :,
                                                                 64:65])
                                trt = [None]
                                for it in range(4):
                                    dst = onTq[:, it * 128 + hi * 64:
                                               it * 128 + hi * 64 + 64]
                                    src = oTt[:, it * 65:it * 65 + 64]
                                    e = bal.pick({"S": _cS(64) + S_RESERVE,
                                                  "D": _cD(64)})
                                    if e == "S":
                                        nc.scalar.mul(dst, src,
                                                      rec[:, it:it + 1])
                                    else:
                                        nc.vector.tensor_scalar(
                                            dst, src, rec[:, it:it + 1],
                                            None, mybir.AluOpType.mult)
                                    if hi == 1 and q > 0:
                                        nc.sync.dma_start_transpose(
                                            onqT[:, it * 128:
                                                 (it + 1) * 128],
                                            onTq[:, it * 128:
                                                 (it + 1) * 128])
                                    elif hi == 1:
                                        # final quarter: PE transpose
                                        # (identity permutation) + engine
                                        # evac -- far lower latency than
                                        # the DMA xbar when no later work
                                        # hides it
                                        if it == 0:
                                            trt[0] = sps.tile(
                                                [128, 1024], F32,
                                                tag="st", name="tr")
                                        trb = trt[0][:, :].bitcast(BF16)
                                        nc.tensor.matmul(
                                            trb[:, it * 128:
                                                (it + 1) * 128],
                                            onTq[:, it * 128:
                                                 (it + 1) * 128],
                                            mask_sb[:, 640:768],
                                            is_transpose=True,
                                            skip_group_check=True)
                                        emit_copy(
                                            onqT[:, it * 128:
                                                 (it + 1) * 128],
                                            trb[:, it * 128:
                                                (it + 1) * 128], 128)
                            return norm

                        norm_fns[hi] = make_norm()
                        for ui, unit in enumerate(units):
                            if USE_FPS_BND and len(unit) == 1 and ui < 8:
                                # boundary singles borrow the chunk bank:
                                # a 4th score slot right at the head start
                                st = fps.tile([128, 512], F32, tag="fp",
                                              name="stb")
                            else:
                                st = sps.tile([128, 1024], F32, tag="st")
                            for k, jb in enumerate(unit):
                                w = _ic(qs, jb)
                                nc.tensor.matmul(
                                    st[:, k * 512:k * 512 + w],
                                    kch[:, jb * 128:(jb + 1) * 128],
                                    qch[:, qs:qs + w],
                                    start=True, stop=True,
                                    skip_group_check=True,
                                )
                            pt = ptp.tile([128, 1024], BF16, tag="pt")
                            if len(unit) == 2:
                                if EXP_SPLIT:
                                    # two half-exps FORCED onto opposite
                                    # engines: the score tile frees at
                                    # max(S, D) half-latency instead of
                                    # one full-pair op
                                    sch_ok = (exp_sch_free[0]
                                              <= SCH_CAP * 146944)
                                    costs = {"S": _cS(512)}
                                    if sch_ok:
                                        costs["D"] = _cD(512)
                                    e0 = bal.pick(costs)
                                    e1 = ("D" if (e0 == "S" and sch_ok)
                                          else "S")
                                    bal.t[e1] += (_cD(512) if e1 == "D"
                                                  else _cS(512))
                                    for k, e in ((0, e0), (1, e1)):
                                        in_ap = st[:, k * 512:
                                                   (k + 1) * 512]
                                        out_ap = pt[:, k * 512:
                                                    (k + 1) * 512]
                                        exp_tot_free[0] += 512
                                        if e == "S":
                                            nc.scalar.activation(
                                                out_ap, in_ap,
                                                mybir.
                                                ActivationFunctionType.Exp,
                                                scale=0.125)
                                        else:
                                            exp_sch_free[0] += 512
                                            nc.vector.tensor_scalar(
                                                out_ap.bitcast(I16),
                                                in_ap, A_SCH, B_SCH,
                                                mybir.AluOpType.mult,
                                                mybir.AluOpType.add)
                                else:
                                    emit_exp(st, pt, 1024, False)
                            else:
                                w = _ic(qs, unit[0])
                                bnd = (w == (unit[0] + 2) * 128 - qs)
                                emit_exp(st, pt, w, bnd)
                            for k, jb in enumerate(unit):
                                pvq.append(
                                    (lambda jb=jb, pt=pt, k=k,
                                     f=emit_pv: f(jb, pt, k), hi))
                            # drip previous quarter's output projection
                            if DRIP_FIRST and ui >= 1:
                                drip()
                            # PV deferred PV_DEFER blocks so the exp latency
                            # hides under the interleaved score matmuls
                            while len(pvq) > PV_DEFER:
                                pop_one()
                            if not DRIP_FIRST and ui >= 1:
                                drip()
                    while pvq:
                        pop_one()
                    for ph in sorted(norm_fns):
                        norm_fns.pop(ph)()
                    queue_p3(q, onqT)

                # slab pipeline from the tail: attention quarter q needs
                # Kc/vr blocks >= 4q-1, i.e. slabs q-1..7, so A(q) runs
                # right after P(q-1) while earlier slabs' DMAs stream.
                # All DMAs are issued up-front in consumption order.
                load_slab(7)
                for name, t in [("WqB", WqB), ("WkA", WkA), ("WkB", WkB),
                                ("Wv", Wv)]:
                    ws = persist.tile([128, E], BF16, tag=name, name=name)
                    nc.sync.dma_start(ws[:, :], t.ap())
                    w_sb[name] = ws
                load_slab(6)
                wo_sb = persist.tile([128, 2 * E], BF16, tag="Wo", name="Wo")
                nc.sync.dma_start(wo_sb[:, :], Wo.ap())
                mask_sb = persist.tile([128, 768], BF16, tag="maskbuf",
                                       name="maskbuf")
                nc.sync.dma_start(mask_sb[:, :], maskbuf.ap())
                mask3_sb = persist.tile([128, 512], F32, tag="mask3",
                                        name="mask3")
                nc.sync.dma_start(mask3_sb[:, :], mask3.ap())
                for sq in range(5, -1, -1):
                    load_slab(sq)
                proj_slab(7)
                proj_slab(6)
                for sq in range(5, -1, -1):
                    attn(sq + 2)
                    proj_slab(sq)
                attn(1)
                attn(0)
                while pending_p3:
                    pending_p3.pop(0)(flush=True)

    _orig_to_json = nc.to_json_bytes

    def _to_json_bytes_split():
        return orjson.dumps(_split_bir_waits(orjson.loads(_orig_to_json())))

    nc.to_json_bytes = _to_json_bytes_split
    return nc


def _build_mask3():
    """Additive Schraudolph mask [128, 512] f32: [ones(384) | tril(128)]
    holding B_SCH where allowed, NEGV where banded-masked; boundary blocks
    window it right-aligned at their width."""
    jj = np.arange(128)[:, None]
    ii = np.arange(128)[None, :]
    trilB = np.where(ii <= jj, B_SCH, NEGV).astype(np.float32)
    ones = np.full((128, 384), B_SCH, np.float32)
    return np.concatenate([ones, trilB], 1)


def shard_inputs(inputs):
    """Build the 8 per-core input maps (host-side layout prep)."""
    q, k, v = inputs["query"], inputs["key"], inputs["value"]
    qWr, qWi = np.asarray(inputs["qWr"]), np.asarray(inputs["qWi"])
    kWr, kWi = np.asarray(inputs["kWr"]), np.asarray(inputs["kWi"])
    vWr = np.asarray(inputs["vWr"])
    qbr, qbi = np.asarray(inputs["qbr"]), np.asarray(inputs["qbi"])
    kbr, kbi = np.asarray(inputs["kbr"]), np.asarray(inputs["kbi"])

    maskb = np.concatenate(
        [np.tril(np.ones((128, 128), np.float32)),
         np.zeros((128, 512), np.float32),
         np.eye(128, dtype=np.float32)], axis=1).astype(BF)
    mask3 = _build_mask3()

    xT = {}
    for b in range(B):
        xT[b] = tuple(
            np.ascontiguousarray(np.asarray(t)[b].T).astype(BF)
            for t in (q, k, v)
        )

    def sb_layout(w):
        # [E, 128] weight -> SBUF layout [128, E] (partition-major e-tiles)
        Ex = w.shape[0]
        return np.ascontiguousarray(
            w.reshape(Ex // 128, 128, 128).transpose(1, 0, 2).reshape(128, Ex))

    def wq(Wr, Wi, h):
        return sb_layout(np.ascontiguousarray(
            np.concatenate([Wr[h * D:(h + 1) * D], Wi[h * D:(h + 1) * D]], 0).T
        ).astype(BF))

    def bias2(br, bi, h):
        return np.concatenate(
            [br[h * D:(h + 1) * D], bi[h * D:(h + 1) * D]]
        ).astype(np.float32)[:, None]

    oWr, oWi = np.asarray(inputs["oWr"]), np.asarray(inputs["oWi"])
    in_maps = []
    for c in range(8):
        b = c // 4
        hA = 2 * (c % 4)
        hB = hA + 1
        qTb, kTb, vTb = xT[b]
        wv = sb_layout(np.ascontiguousarray(np.concatenate(
            [vWr[hA * D:(hA + 1) * D], vWr[hB * D:(hB + 1) * D]], 0).T
        ).astype(BF))
        wo_r = np.concatenate(
            [oWr[:, hA * D:(hA + 1) * D], oWr[:, hB * D:(hB + 1) * D]], 1).T
        wo_i = np.concatenate(
            [oWi[:, hA * D:(hA + 1) * D], oWi[:, hB * D:(hB + 1) * D]], 1).T
        wo = np.ascontiguousarray(
            np.concatenate([wo_r, wo_i], 1)).astype(BF)
        in_maps.append({
            "qT": qTb, "kT": kTb, "vT": vTb,
            "WqA": wq(qWr, qWi, hA), "WqB": wq(qWr, qWi, hB),
            "WkA": wq(kWr, kWi, hA), "WkB": wq(kWr, kWi, hB),
            "Wv": wv, "Wo": wo,
            "bqA": bias2(qbr, qbi, hA), "bqB": bias2(qbr, qbi, hB),
            "bkA": bias2(kbr, kbi, hA), "bkB": bias2(kbr, kbi, hB),
            "maskbuf": maskb, "mask3": mask3,
        })
    return in_maps


_NC_CACHE = None


def kernel(**inputs):
    global _NC_CACHE
    if _NC_CACHE is None:
        _NC_CACHE = build_program()
    nc = _NC_CACHE
    in_maps = shard_inputs(inputs)
    res = run_bass_kernel_spmd(nc, in_maps, core_ids=list(range(8)))
    obr = np.asarray(inputs["obr"], np.float32)
    obi = np.asarray(inputs["obi"], np.float32)
    oWr = np.asarray(inputs["oWr"], np.float32)
    oWi = np.asarray(inputs["oWi"], np.float32)
    vbr = np.asarray(inputs["vbr"], np.float32)
    acc = np.zeros((B, L, 2 * E), np.float32)
    for c in range(8):
        acc[c // 4] += np.asarray(res.results[c]["out"], np.float32)
    # V-projection bias folded through the output projection (host-side)
    out_r = acc[:, :, :E] + obr + oWr @ vbr
    out_i = acc[:, :, E:] + obi + oWi @ vbr
    return out_r, out_i
